# revision 7
# baseline (speedup 1.0000x reference)
"""BERT per-word mean-pool (segment reduce) on 8 Trainium2 NeuronCores.

Problem: output[B=64, S=512, E=768] f32, mappings[B, W=255] int32 (values 1 or 2).
Per sentence, strip [CLS]/[SEP], mean-pool contiguous BPE spans into word vectors.

Key identity: every word's span has 1 or 2 BPE rows.  With s = span start and
e = span end (exclusive) inside the stripped sequence, the mean is ALWAYS
    out[w] = (hs[s] + hs[e-1]) * 0.5
because for a 1-token span s == e-1 and (x + x)/2 == x exactly in f32.
So the whole kernel is two row-gathers, an add, and a scale by 0.5.

Sharding: pure data parallel, 8 sentences per core, no cross-core comms.
Device work per core: dma_gather 2x2048 rows of 3KB (12.6 MB), DVE add,
ACT scale, contiguous store (6.3 MB) -> memory-bound at ~360 GB/s.

Uses the InstDMAGatherAnt custom SWDGE gather (mlp ucode library,
auto-loaded by Bacc.compile) -- the production-proven gather path; raw
indirect InstDMACopy corrupts when two indirect DMAs are in flight.
"""

import numpy as np

from concourse import bacc, mybir, tile
from concourse.bass_utils import run_bass_kernel_spmd

B, S, W, E = 64, 512, 255, 768
NCORES = 8
BPC = B // NCORES            # sentences per core
NW = BPC * W                 # 2040 real words per core
NWP = 2048                   # padded word count (multiple of 512)
NCHUNK = 4                   # chunks per core
CPW = NWP // NCHUNK          # 512 words per chunk
JJ = CPW // 128              # 4 words per partition per chunk
ROWS = BPC * S               # 4096 input rows per core
NIDX = 2 * CPW               # 1024 gather indices per chunk (A then B)

_F32 = mybir.dt.float32
_I16 = mybir.dt.int16


def _build_nc(reps=1):
    nc = bacc.Bacc(
        "TRN2", target_bir_lowering=False, debug=False, num_devices=NCORES
    )
    x = nc.dram_tensor("x", [ROWS, E], _F32, kind="ExternalInput").ap()
    # per chunk: 1024 int16 indices, wrapped [i%16, i//16] into 16 partitions
    # and replicated 8x down to 128 partitions (Q7 core replication).
    idx = nc.dram_tensor(
        "idx", [NCHUNK, 128, NIDX // 16], _I16, kind="ExternalInput"
    ).ap()
    y = nc.dram_tensor("y", [NWP, E], _F32, kind="ExternalOutput").ap()

    with tile.TileContext(nc) as tc:
        with (
            tc.tile_pool(name="idxp", bufs=1) as ipool,
            tc.tile_pool(name="io", bufs=2) as pool,
        ):
            its = []
            for q in range(NCHUNK):
                it = ipool.tile([128, NIDX // 16], _I16, tag=f"it{q}")
                nc.sync.dma_start(out=it[:], in_=idx[q])
                its.append(it)
            for _rep in range(reps):
                for q in range(NCHUNK):
                    # gathered slot i -> T[i % 128, i // 128, :]
                    # i = c*128 + p:  c in 0..3 -> first-BPE row of word
                    # w = q*512 + p*4 + c;  c in 4..7 -> last-BPE row of it.
                    t = pool.tile([128, 2 * JJ * E], _F32, tag="t")
                    nc.gpsimd.dma_gather(
                        t[:].rearrange("p (c e) -> p c e", e=E),
                        x[:, :],
                        its[q][:],
                        NIDX,
                        NIDX,
                        E,
                    )
                    c = pool.tile([128, JJ * E], _F32, tag="c")
                    nc.vector.tensor_add(
                        out=c[:], in0=t[:, : JJ * E], in1=t[:, JJ * E :]
                    )
                    nc.scalar.mul(c[:], c[:], 0.5)
                    nc.sync.dma_start(
                        out=y[q * CPW : (q + 1) * CPW, :].rearrange(
                            "(p j) e -> p (j e)", p=128
                        ),
                        in_=c[:],
                    )
    nc.compile()
    return nc


_NC = {}


def _get_nc(reps=1):
    if reps not in _NC:
        _NC[reps] = _build_nc(reps)
    return _NC[reps]


def _make_in_maps(output, mappings):
    output = np.ascontiguousarray(np.asarray(output), dtype=np.float32)
    mappings = np.asarray(mappings, dtype=np.int32)
    ends = np.cumsum(mappings, axis=1, dtype=np.int32)  # [B, W] exclusive ends
    src_a = ends - mappings + 1                         # +1: skip [CLS]
    src_b = ends                                        # (e-1) + 1

    in_maps = []
    for k in range(NCORES):
        bs = slice(k * BPC, (k + 1) * BPC)
        base = (np.arange(BPC, dtype=np.int32) * S)[:, None]
        a = (src_a[bs] + base).reshape(-1)
        b = (src_b[bs] + base).reshape(-1)
        pad = np.zeros(NWP - NW, np.int32)
        a = np.concatenate([a, pad])  # [NWP] word-ordered flat row ids
        b = np.concatenate([b, pad])
        idx = np.empty((NCHUNK, 128, NIDX // 16), np.int16)
        for q in range(NCHUNK):
            aq = a[q * CPW : (q + 1) * CPW].reshape(128, JJ)  # [p, c]
            bq = b[q * CPW : (q + 1) * CPW].reshape(128, JJ)
            flat = np.concatenate(
                [aq.T.ravel(), bq.T.ravel()]
            )  # gathered order i = c*128 + p, A then B
            idx[q] = np.tile(flat.reshape(NIDX // 16, 16).T, (8, 1))
        x = np.ascontiguousarray(output[bs].reshape(ROWS, E))
        in_maps.append({"x": x, "idx": idx})
    return in_maps


def _run(output, mappings, reps=1, **kw):
    in_maps = _make_in_maps(output, mappings)
    nc = _get_nc(reps)
    res = run_bass_kernel_spmd(nc, in_maps, list(range(NCORES)), **kw)
    outs = [r["y"][:NW].reshape(BPC, W, E) for r in res.results]
    return np.concatenate(outs, axis=0), res


def kernel(output, mappings):
    full, _ = _run(output, mappings)
    return full


# revision 14
# speedup vs baseline: 40.4413x; 40.4413x over previous
"""BERT per-word mean-pool (segment reduce) on 8 Trainium2 NeuronCores.

Problem: output[B=64, S=512, E=768] f32, mappings[B, W=255] int32 (values 1 or 2).
Per sentence, strip [CLS]/[SEP], mean-pool contiguous BPE spans into word vectors.

Key identity: every word's span has 1 or 2 BPE rows.  With s = span start and
e = span end (exclusive) inside the stripped sequence, the mean is ALWAYS
    out[w] = (hs[s] + hs[e-1]) * 0.5
because for a 1-token span s == e-1 and (x + x)/2 == x exactly in f32.
So the whole kernel is two row-gathers, an add, and a scale by 0.5.

Sharding: pure data parallel, 8 sentences per core, no cross-core comms.
Device work per core: dma_gather 2x2048 rows of 3KB (12.6 MB), DVE add,
ACT scale, contiguous store (6.3 MB) -> memory-bound at ~360 GB/s.

Uses the InstDMAGatherAnt custom SWDGE gather (mlp ucode library,
auto-loaded by Bacc.compile) -- the production-proven gather path; raw
indirect InstDMACopy corrupts when two indirect DMAs are in flight.
"""

import numpy as np

from concourse import bacc, mybir, tile
from concourse.bass_utils import run_bass_kernel_spmd

B, S, W, E = 64, 512, 255, 768
NCORES = 8
BPC = B // NCORES            # sentences per core
NW = BPC * W                 # 2040 real words per core
NWP = 2048                   # padded word count (multiple of 512)
NCHUNK = 4                   # chunks per core
CPW = NWP // NCHUNK          # 512 words per chunk
JJ = CPW // 128              # 4 words per partition per chunk
ROWS = BPC * S               # 4096 input rows per core
NIDX = 2 * CPW               # 1024 gather indices per chunk (A then B)

_F32 = mybir.dt.float32
_I16 = mybir.dt.int16


def _build_nc(reps=1, bufs=2, order="pc"):
    nc = bacc.Bacc(
        "TRN2", target_bir_lowering=False, debug=False, num_devices=NCORES
    )
    x = nc.dram_tensor("x", [ROWS, E], _F32, kind="ExternalInput").ap()
    # per chunk: 1024 int16 indices, wrapped [i%16, i//16] into 16 partitions
    # and replicated 8x down to 128 partitions (Q7 core replication).
    idx = nc.dram_tensor(
        "idx", [NCHUNK, 128, NIDX // 16], _I16, kind="ExternalInput"
    ).ap()
    y = nc.dram_tensor("y", [NWP, E], _F32, kind="ExternalOutput").ap()

    with tile.TileContext(nc) as tc:
        with (
            tc.tile_pool(name="idxp", bufs=1) as ipool,
            tc.tile_pool(name="io", bufs=bufs) as pool,
        ):
            its = []
            for q in range(NCHUNK):
                it = ipool.tile([128, NIDX // 16], _I16, tag=f"it{q}")
                nc.sync.dma_start(out=it[:], in_=idx[q])
                its.append(it)
            for _rep in range(reps):
                for q in range(NCHUNK):
                    # gathered slot i -> T[i % 128, i // 128, :]
                    # i = c*128 + p:  c in 0..3 -> first-BPE row of word
                    # w = q*512 + p*4 + c;  c in 4..7 -> last-BPE row of it.
                    t = pool.tile([128, 2 * JJ * E], _F32, tag="t")
                    nc.gpsimd.dma_gather(
                        t[:].rearrange("p (c e) -> p c e", e=E),
                        x[:, :],
                        its[q][:],
                        NIDX,
                        NIDX,
                        E,
                    )
                    c = pool.tile([128, JJ * E], _F32, tag="c")
                    nc.vector.tensor_add(
                        out=c[:], in0=t[:, : JJ * E], in1=t[:, JJ * E :]
                    )
                    nc.scalar.mul(c[:], c[:], 0.5)
                    ychunk = y[q * CPW : (q + 1) * CPW, :]
                    if order == "pc":
                        nc.sync.dma_start(
                            out=ychunk.rearrange("(p j) e -> p (j e)", p=128),
                            in_=c[:],
                        )
                    else:
                        nc.sync.dma_start(
                            out=ychunk.rearrange("(j p) e -> p j e", p=128),
                            in_=c[:].rearrange("p (j e) -> p j e", e=E),
                        )
    nc.compile()
    return nc


_NC = {}


def _get_nc(reps=1, bufs=2, order="pc"):
    key = (reps, bufs, order)
    if key not in _NC:
        _NC[key] = _build_nc(reps, bufs, order)
    return _NC[key]


def _make_in_maps(output, mappings, order="pc"):
    output = np.ascontiguousarray(np.asarray(output), dtype=np.float32)
    mappings = np.asarray(mappings, dtype=np.int32)
    ends = np.cumsum(mappings, axis=1, dtype=np.int32)  # [B, W] exclusive ends
    src_a = ends - mappings + 1                         # +1: skip [CLS]
    src_b = ends                                        # (e-1) + 1

    in_maps = []
    for k in range(NCORES):
        bs = slice(k * BPC, (k + 1) * BPC)
        base = (np.arange(BPC, dtype=np.int32) * S)[:, None]
        a = (src_a[bs] + base).reshape(-1)
        b = (src_b[bs] + base).reshape(-1)
        pad = np.zeros(NWP - NW, np.int32)
        a = np.concatenate([a, pad])  # [NWP] word-ordered flat row ids
        b = np.concatenate([b, pad])
        idx = np.empty((NCHUNK, 128, NIDX // 16), np.int16)
        for q in range(NCHUNK):
            aq = a[q * CPW : (q + 1) * CPW]
            bq = b[q * CPW : (q + 1) * CPW]
            if order == "pc":
                # gathered i = c*128 + p holds word q*512 + p*JJ + c
                aq = aq.reshape(128, JJ).T.ravel()
                bq = bq.reshape(128, JJ).T.ravel()
            # order 'seq': gathered i holds word q*512 + i (ascending rows)
            flat = np.concatenate([aq, bq])
            idx[q] = np.tile(flat.reshape(NIDX // 16, 16).T, (8, 1))
        x = np.ascontiguousarray(output[bs].reshape(ROWS, E))
        in_maps.append({"x": x, "idx": idx})
    return in_maps


def _run(output, mappings, reps=1, bufs=2, order="pc", **kw):
    in_maps = _make_in_maps(output, mappings, order)
    nc = _get_nc(reps, bufs, order)
    res = run_bass_kernel_spmd(nc, in_maps, list(range(NCORES)), **kw)
    outs = [r["y"][:NW].reshape(BPC, W, E) for r in res.results]
    return np.concatenate(outs, axis=0), res


def kernel(output, mappings):
    full, _ = _run(output, mappings)
    return full


# revision 17
# speedup vs baseline: 41.2587x; 1.0202x over previous
"""BERT per-word mean-pool (segment reduce) on 8 Trainium2 NeuronCores.

Problem: output[B=64, S=512, E=768] f32, mappings[B, W=255] int32 (values 1 or 2).
Per sentence, strip [CLS]/[SEP], mean-pool contiguous BPE spans into word vectors.

Key identity: every word's span has 1 or 2 BPE rows.  With s = span start and
e = span end (exclusive) inside the stripped sequence, the mean is ALWAYS
    out[w] = (hs[s] + hs[e-1]) * 0.5
because for a 1-token span s == e-1 and (x + x)/2 == x exactly in f32.
So the whole kernel is two row-gathers, an add, and a scale by 0.5.

Sharding: pure data parallel, 8 sentences per core, no cross-core comms.
Device work per core: dma_gather 2x2048 rows of 3KB (12.6 MB), DVE add,
ACT scale, contiguous store (6.3 MB) -> memory-bound at ~360 GB/s.

Uses the InstDMAGatherAnt custom SWDGE gather (mlp ucode library,
auto-loaded by Bacc.compile) -- the production-proven gather path; raw
indirect InstDMACopy corrupts when two indirect DMAs are in flight.
"""

import numpy as np

from concourse import bacc, mybir, tile
from concourse.bass_utils import run_bass_kernel_spmd

B, S, W, E = 64, 512, 255, 768
NCORES = 8
BPC = B // NCORES            # sentences per core
NW = BPC * W                 # 2040 real words per core
NWP = 2048                   # padded word count (multiple of 512)
NCHUNK = 4                   # chunks per core
CPW = NWP // NCHUNK          # 512 words per chunk
JJ = CPW // 128              # 4 words per partition per chunk
ROWS = BPC * S               # 4096 input rows per core
NIDX = 2 * CPW               # 1024 gather indices per chunk (A then B)

_F32 = mybir.dt.float32
_I16 = mybir.dt.int16


def _build_nc(reps=1, bufs=2, order="pc", nq=1):
    nc = bacc.Bacc(
        "TRN2",
        target_bir_lowering=False,
        debug=False,
        num_devices=NCORES,
        num_swdge_queues=nq,
    )
    x = nc.dram_tensor("x", [ROWS, E], _F32, kind="ExternalInput").ap()
    # per chunk: 1024 int16 indices, wrapped [i%16, i//16] into 16 partitions
    # and replicated 8x down to 128 partitions (Q7 core replication).
    idx = nc.dram_tensor(
        "idx", [NCHUNK, 128, NIDX // 16], _I16, kind="ExternalInput"
    ).ap()
    y = nc.dram_tensor("y", [NWP, E], _F32, kind="ExternalOutput").ap()

    with tile.TileContext(nc) as tc:
        with (
            tc.tile_pool(name="idxp", bufs=1) as ipool,
            tc.tile_pool(name="io", bufs=bufs) as pool,
        ):
            its = []
            for q in range(NCHUNK):
                it = ipool.tile([128, NIDX // 16], _I16, tag=f"it{q}")
                nc.sync.dma_start(out=it[:], in_=idx[q])
                its.append(it)
            for _rep in range(reps):
                for q in range(NCHUNK):
                    # gathered slot i -> T[i % 128, i // 128, :]
                    # i = c*128 + p:  c in 0..3 -> first-BPE row of word
                    # w = q*512 + p*4 + c;  c in 4..7 -> last-BPE row of it.
                    t = pool.tile([128, 2 * JJ * E], _F32, tag="t")
                    nc.gpsimd.dma_gather(
                        t[:].rearrange("p (c e) -> p c e", e=E),
                        x[:, :],
                        its[q][:],
                        NIDX,
                        NIDX,
                        E,
                        queue_num=q % nq,
                    )
                    c = pool.tile([128, JJ * E], _F32, tag="c")
                    nc.vector.tensor_add(
                        out=c[:], in0=t[:, : JJ * E], in1=t[:, JJ * E :]
                    )
                    nc.scalar.mul(c[:], c[:], 0.5)
                    ychunk = y[q * CPW : (q + 1) * CPW, :]
                    if order == "pc":
                        nc.sync.dma_start(
                            out=ychunk.rearrange("(p j) e -> p (j e)", p=128),
                            in_=c[:],
                        )
                    else:
                        nc.sync.dma_start(
                            out=ychunk.rearrange("(j p) e -> p j e", p=128),
                            in_=c[:].rearrange("p (j e) -> p j e", e=E),
                        )
    nc.compile()
    return nc


_NC = {}


def _get_nc(reps=1, bufs=2, order="pc", nq=1):
    key = (reps, bufs, order, nq)
    if key not in _NC:
        _NC[key] = _build_nc(reps, bufs, order, nq)
    return _NC[key]


def _make_in_maps(output, mappings, order="pc"):
    output = np.ascontiguousarray(np.asarray(output), dtype=np.float32)
    mappings = np.asarray(mappings, dtype=np.int32)
    ends = np.cumsum(mappings, axis=1, dtype=np.int32)  # [B, W] exclusive ends
    src_a = ends - mappings + 1                         # +1: skip [CLS]
    src_b = ends                                        # (e-1) + 1

    in_maps = []
    for k in range(NCORES):
        bs = slice(k * BPC, (k + 1) * BPC)
        base = (np.arange(BPC, dtype=np.int32) * S)[:, None]
        a = (src_a[bs] + base).reshape(-1)
        b = (src_b[bs] + base).reshape(-1)
        pad = np.zeros(NWP - NW, np.int32)
        a = np.concatenate([a, pad])  # [NWP] word-ordered flat row ids
        b = np.concatenate([b, pad])
        idx = np.empty((NCHUNK, 128, NIDX // 16), np.int16)
        for q in range(NCHUNK):
            aq = a[q * CPW : (q + 1) * CPW]
            bq = b[q * CPW : (q + 1) * CPW]
            if order == "pc":
                # gathered i = c*128 + p holds word q*512 + p*JJ + c
                aq = aq.reshape(128, JJ).T.ravel()
                bq = bq.reshape(128, JJ).T.ravel()
            # order 'seq': gathered i holds word q*512 + i (ascending rows)
            flat = np.concatenate([aq, bq])
            idx[q] = np.tile(flat.reshape(NIDX // 16, 16).T, (8, 1))
        x = np.ascontiguousarray(output[bs].reshape(ROWS, E))
        in_maps.append({"x": x, "idx": idx})
    return in_maps


def _run(output, mappings, reps=1, bufs=2, order="pc", nq=1, **kw):
    in_maps = _make_in_maps(output, mappings, order)
    nc = _get_nc(reps, bufs, order, nq)
    res = run_bass_kernel_spmd(nc, in_maps, list(range(NCORES)), **kw)
    outs = [r["y"][:NW].reshape(BPC, W, E) for r in res.results]
    return np.concatenate(outs, axis=0), res


def kernel(output, mappings):
    full, _ = _run(output, mappings)
    return full
